# revision 1
# baseline (speedup 1.0000x reference)
"""AgentAttention kernel for 8 Trainium2 NeuronCores.

Data-parallel over batch (b=16 -> 2 per core), params/biases replicated.
All shape/layout constants are hardcoded to the problem spec:
  x: (16, 3, 64, 64) f32 -> out: (16, 512, 64, 64) f32
"""

import numpy as np
import jax
import jax.numpy as jnp

B, CIN, H, W = 16, 3, 64, 64
C, NH, AGENT, POOL = 512, 16, 49, 7
N = H * W
HD = C // NH
SCALE = HD ** -0.5
NDEV = 8
BPD = B // NDEV  # batches per device

_HIGH = jax.lax.Precision.HIGHEST


def _pool_matrix():
    """(AGENT, N) matrix M with qt[b,a,c] = sum_t M[a,t] q[b,t,c].

    Matches reference: q.reshape(b, h, w, C) treats token t as grid
    (t // W, t % W); adaptive_avg_pool2d(64->7) uses floor/ceil ranges.
    """
    M = np.zeros((AGENT, N), dtype=np.float32)
    starts = [int(np.floor(i * H / POOL)) for i in range(POOL)]
    ends = [int(np.ceil((i + 1) * H / POOL)) for i in range(POOL)]
    for i in range(POOL):
        ri = range(starts[i], ends[i])
        for j in range(POOL):
            cj = range(starts[j], ends[j])
            val = 1.0 / (len(ri) * len(cj))
            a = i * POOL + j
            for r in ri:
                for c in cj:
                    M[a, r * W + c] = val
    return M


def _bilinear_resize_np(img, out_h, out_w):
    """numpy copy of reference bilinear_resize (align_corners=False)."""
    img = np.asarray(img, dtype=np.float32)
    Hi, Wi = img.shape[-2], img.shape[-1]

    def grid(out, size):
        c = (np.arange(out, dtype=np.float32) + 0.5) * (size / out) - 0.5
        c = np.clip(c, 0.0, size - 1.0)
        i0 = np.floor(c).astype(np.int32)
        i1 = np.minimum(i0 + 1, size - 1)
        wgt = (c - i0.astype(np.float32)).astype(np.float32)
        return i0, i1, wgt

    h0, h1, wh = grid(out_h, Hi)
    w0, w1, ww = grid(out_w, Wi)
    rows = img[..., h0, :] * (1.0 - wh)[:, None] + img[..., h1, :] * wh[:, None]
    return rows[..., w0] * (1.0 - ww) + rows[..., w1] * ww


def _device_fn(x, in_w, in_b, qkv_w, qkv_b, proj_w, proj_b,
               dwc_w, dwc_b, Mpool, bias1, bias2):
    """Per-device compute on a (BPD, CIN, H, W) shard."""
    b = BPD
    # token order t = w*H + h  (matches reference transpose(0,3,2,1))
    xt = jnp.transpose(x, (0, 3, 2, 1)).reshape(b, N, CIN)
    xf = jnp.einsum('bni,io->bno', xt, in_w, precision=_HIGH) + in_b

    qkv = jnp.einsum('bnc,co->bno', xf, qkv_w, precision=_HIGH) + qkv_b
    q = qkv[:, :, :C]
    k = qkv[:, :, C:2 * C]
    v = qkv[:, :, 2 * C:]

    # agent tokens via pooling matrix
    qt = jnp.einsum('an,bnc->bac', Mpool, q, precision=_HIGH)  # (b, AGENT, C)

    qh = q.reshape(b, N, NH, HD).transpose(0, 2, 1, 3)      # (b, nh, n, hd)
    kh = k.reshape(b, N, NH, HD).transpose(0, 2, 1, 3)
    vh = v.reshape(b, N, NH, HD).transpose(0, 2, 1, 3)
    qth = qt.reshape(b, AGENT, NH, HD).transpose(0, 2, 1, 3)  # (b, nh, A, hd)

    # stage 1: agent -> tokens
    logits1 = jnp.einsum('bhad,bhnd->bhan', qth * SCALE, kh,
                         precision=_HIGH) + bias1[None]
    attn1 = jax.nn.softmax(logits1, axis=-1)
    agent_v = jnp.einsum('bhan,bhnd->bhad', attn1, vh, precision=_HIGH)

    # stage 2: tokens -> agents
    logits2 = jnp.einsum('bhnd,bhad->bhna', qh * SCALE, qth,
                         precision=_HIGH) + bias2[None]
    attn2 = jax.nn.softmax(logits2, axis=-1)
    out = jnp.einsum('bhna,bhad->bhnd', attn2, agent_v, precision=_HIGH)
    out = out.transpose(0, 2, 1, 3).reshape(b, N, C)

    # depthwise 3x3 conv residual on v (padding 1), via 9 shifted taps
    v_img = vh.transpose(0, 2, 1, 3).reshape(b, H, W, C)     # (b, h, w, C)
    vp = jnp.pad(v_img, ((0, 0), (1, 1), (1, 1), (0, 0)))
    dw = jnp.zeros_like(v_img)
    for di in range(3):
        for dj in range(3):
            tap = dwc_w[:, 0, di, dj]                         # (C,)
            dw = dw + vp[:, di:di + H, dj:dj + W, :] * tap
    dw = dw + dwc_b
    out = out + dw.reshape(b, N, C)

    out = jnp.einsum('bnc,co->bno', out, proj_w, precision=_HIGH) + proj_b
    return jnp.transpose(out.reshape(b, H, W, C), (0, 3, 1, 2))


_pmapped = None


def _get_pmapped():
    global _pmapped
    if _pmapped is None:
        _pmapped = jax.pmap(
            _device_fn,
            in_axes=(0,) + (None,) * 11,
            devices=jax.devices()[:NDEV],
        )
    return _pmapped


def kernel(x, in_w, in_b, qkv_w, qkv_b, proj_w, proj_b, dwc_w, dwc_b,
           an_bias, na_bias, ah_bias, aw_bias, ha_bias, wa_bias):
    x = np.asarray(x, dtype=np.float32)

    # precompute attention bias tables (exactly as reference, in fp32 numpy)
    Mpool = _pool_matrix()
    pb1 = _bilinear_resize_np(np.asarray(an_bias), H, W).reshape(NH, AGENT, N)
    ah = np.asarray(ah_bias, dtype=np.float32)
    aw = np.asarray(aw_bias, dtype=np.float32)
    pb2 = (ah + aw).reshape(NH, AGENT, N)
    bias1 = (pb1 + pb2).astype(np.float32)                    # (nh, A, n)

    nb1 = _bilinear_resize_np(np.asarray(na_bias), H, W).reshape(NH, AGENT, N)
    nb1 = np.transpose(nb1, (0, 2, 1))                        # (nh, n, A)
    ha = np.asarray(ha_bias, dtype=np.float32)
    wa = np.asarray(wa_bias, dtype=np.float32)
    nb2 = (ha + wa).reshape(NH, N, AGENT)
    bias2 = (nb1 + nb2).astype(np.float32)                    # (nh, n, A)

    xs = x.reshape(NDEV, BPD, CIN, H, W)
    out = _get_pmapped()(
        xs,
        jnp.asarray(in_w), jnp.asarray(in_b),
        jnp.asarray(qkv_w), jnp.asarray(qkv_b),
        jnp.asarray(proj_w), jnp.asarray(proj_b),
        jnp.asarray(dwc_w), jnp.asarray(dwc_b),
        jnp.asarray(Mpool), jnp.asarray(bias1), jnp.asarray(bias2),
    )
    out = np.asarray(out).reshape(B, C, H, W).astype(np.float32)
    return out


if __name__ == "__main__":
    rng = np.random.RandomState(0)
    demo = {
        "x": rng.randn(B, CIN, H, W).astype(np.float32),
        "in_w": rng.randn(CIN, C).astype(np.float32) * 0.3,
        "in_b": rng.randn(C).astype(np.float32) * 0.02,
        "qkv_w": rng.randn(C, 3 * C).astype(np.float32) * 0.02,
        "qkv_b": np.zeros(3 * C, np.float32),
        "proj_w": rng.randn(C, C).astype(np.float32) * 0.02,
        "proj_b": np.zeros(C, np.float32),
        "dwc_w": rng.randn(C, 1, 3, 3).astype(np.float32) * 0.1,
        "dwc_b": np.zeros(C, np.float32),
        "an_bias": rng.randn(NH, AGENT, 7, 7).astype(np.float32) * 0.02,
        "na_bias": rng.randn(NH, AGENT, 7, 7).astype(np.float32) * 0.02,
        "ah_bias": rng.randn(1, NH, AGENT, H, 1).astype(np.float32) * 0.02,
        "aw_bias": rng.randn(1, NH, AGENT, 1, W).astype(np.float32) * 0.02,
        "ha_bias": rng.randn(1, NH, H, 1, AGENT).astype(np.float32) * 0.02,
        "wa_bias": rng.randn(1, NH, 1, W, AGENT).astype(np.float32) * 0.02,
    }
    y = kernel(**demo)
    print(y.shape, y.dtype)
